# revision 24
# baseline (speedup 1.0000x reference)
"""Trainium2 Bass kernel for BlockIndexNet (per-species MLP over atom blocks).

Strategy: one species block per NeuronCore (8 blocks, 8 cores, data-parallel).
The host gathers each species' atom embeddings via block_index, transposes to
[D_IN, BLOCK] (so the device needs no on-chip transposes: the contraction dim
lands on SBUF partitions for every layer), and zero-pads atoms to a multiple
of the tile size. The device runs a 3-layer MLP with activations kept
transposed ([feature, atom]); matmuls run in float16 (same 10-bit mantissa as
TF32, but 2-byte operands keep the PE weight-load path fast); SiLU runs on the
scalar engine straight out of PSUM. The emission order software-pipelines the
three layers across tiles so the tensor and scalar engines both stay dense.
Output comes back as [D_OUT, B_PAD] per core and is un-transposed and
scattered on the host.
"""

import sys

if "/opt/trn_rl_repo" not in sys.path:
    sys.path.insert(0, "/opt/trn_rl_repo")

import numpy as np

N_ATOMS = 200_000
D_IN = 256
H = 256
D_OUT = 128
N_SPECIES = 8
BLOCK = N_ATOMS // N_SPECIES  # 25000

B_TILE = 512
N_TILES = 49
B_PAD = B_TILE * N_TILES  # 25088

_P = 128
_KC = D_IN // _P  # 2 contraction chunks per layer

# Tuning knobs (test.py may override before first call; defaults are the
# shipped configuration).
WEIGHT_DT = "f16"  # "f32r" | "bf16" | "f16" — dtype of the stationary matmul operand
ACT_DT = "f16"     # "f32r" | "bf16" | "f16" — dtype of the moving matmul operand
WARM_MMS = 8       # dummy matmuls at kernel start to warm the PE HAM clock gate
PS1_BUFS = 2
PS2_BUFS = 2

_program_cache: dict = {}


def _np_dtype(name):
    if name == "bf16":
        import ml_dtypes

        return ml_dtypes.bfloat16
    if name == "f16":
        return np.float16
    return np.float32


def _weight_np_dtype():
    return _np_dtype(WEIGHT_DT)


def _build_program(zero_bias: bool):
    import concourse.bacc as bacc
    import concourse.mybir as mybir
    from concourse.tile import TileContext

    f32 = mybir.dt.float32
    f32r = mybir.dt.float32r
    _dtmap = {"f32r": f32r, "bf16": mybir.dt.bfloat16, "f16": mybir.dt.float16}
    w_dt = _dtmap[WEIGHT_DT]
    a_dt = _dtmap[ACT_DT]
    SILU = mybir.ActivationFunctionType.Silu

    nc = bacc.Bacc("TRN2", num_devices=N_SPECIES)

    xt_d = nc.dram_tensor("xt", [D_IN, B_PAD], a_dt, kind="ExternalInput")
    w1_d = nc.dram_tensor("w1", [D_IN, H], w_dt, kind="ExternalInput")
    w2_d = nc.dram_tensor("w2", [H, H], w_dt, kind="ExternalInput")
    w3_d = nc.dram_tensor("w3", [H, D_OUT], w_dt, kind="ExternalInput")
    if not zero_bias:
        b1_d = nc.dram_tensor("b1", [H], f32, kind="ExternalInput")
        b2_d = nc.dram_tensor("b2", [H], f32, kind="ExternalInput")
        b3_d = nc.dram_tensor("b3", [D_OUT], f32, kind="ExternalInput")
    yt_d = nc.dram_tensor("yt", [D_OUT, B_PAD], f32, kind="ExternalOutput")

    xt_v = xt_d.rearrange("(kc p) n -> p kc n", p=_P)

    with TileContext(nc) as tc:
        with (
            tc.tile_pool(name="wpool", bufs=1) as wpool,
            tc.tile_pool(name="xpool", bufs=8) as xpool,
            tc.tile_pool(name="h1pool", bufs=6) as h1pool,
            tc.tile_pool(name="h2pool", bufs=6) as h2pool,
            tc.tile_pool(name="opool", bufs=6) as opool,
            tc.tile_pool(name="ps1p", bufs=PS1_BUFS, space="PSUM") as ps1p,
            tc.tile_pool(name="ps2p", bufs=PS2_BUFS, space="PSUM") as ps2p,
        ):
            # Warm-up: preload the SILU activation table and keep the PE
            # busy during the input-DMA latency so the HAM clock gate is at
            # full rate when real matmuls arrive. Operands are uninitialized
            # scratch; results are never read.
            warm_sb = wpool.tile([_P, B_TILE], a_dt, tag="warm")
            warm_out = wpool.tile([_P, 16], a_dt, tag="warm_out")
            nc.vector.memset(warm_sb[:], 0.0)
            nc.scalar.activation(warm_out[:], warm_sb[:, :16], SILU)
            warm_ps = ps2p.tile([_P, 2, B_TILE], f32, tag="ps2", name="warm_ps")
            for _ in range(WARM_MMS):
                nc.tensor.matmul(
                    warm_ps[:, 0, :], warm_sb[:, :_P], warm_sb[:], start=True, stop=True
                )

            w1_sb = wpool.tile([_P, _KC, H], w_dt, tag="w1")
            w2_sb = wpool.tile([_P, _KC, H], w_dt, tag="w2")
            w3_sb = wpool.tile([_P, _KC, D_OUT], w_dt, tag="w3")
            if not zero_bias:
                b1_sb = wpool.tile([_P, 2], f32, tag="b1")
                b2_sb = wpool.tile([_P, 2], f32, tag="b2")
                b3_sb = wpool.tile([_P, 1], f32, tag="b3")

            def load_weights():
                nc.sync.dma_start(
                    w1_sb[:], w1_d.rearrange("(kc p) m -> p kc m", p=_P)
                )
                nc.sync.dma_start(
                    w2_sb[:], w2_d.rearrange("(kc p) m -> p kc m", p=_P)
                )
                nc.sync.dma_start(
                    w3_sb[:], w3_d.rearrange("(kc p) m -> p kc m", p=_P)
                )
                if not zero_bias:
                    nc.sync.dma_start(
                        b1_sb[:], b1_d.rearrange("(hh p) -> p hh", p=_P)
                    )
                    nc.sync.dma_start(
                        b2_sb[:], b2_d.rearrange("(hh p) -> p hh", p=_P)
                    )
                    nc.sync.dma_start(
                        b3_sb[:], b3_d.rearrange("(hh p) -> p hh", p=_P)
                    )

            # Software-pipelined stages, skewed so every engine-stream
            # consumer sits behind a producer that has had a full period to
            # finish (avoids head-of-line blocking in the per-engine FIFOs);
            # iteration t emits
            #   dma_x(t+4) | mm1(t+1) | silu2(t-1) | silu1(t+1) | mm2(t)
            #   | mm3(t-1) | bias3/copy(t-1) | dma_out(t-1)
            # Inputs are fetched at pair granularity (1 MiB transfers); the
            # L3 matmul reuses the drained first bank of its tile's ps2
            # allocation, so the two PSUM pools (4 + 4 banks) fill PSUM
            # exactly.
            xts = {}
            ps1s = {}
            ps2s = {}
            h1s = {}
            h2s = {}

            def dma_x(t):
                # pair-granular load: even t loads tiles t and t+1 in one
                # 1 MiB transfer; odd t aliases the even tile's second half.
                if t % 2 == 1:
                    return
                g = t // 2
                n = min(2 * B_TILE, B_PAD - t * B_TILE)
                xts[g] = xpool.tile([_P, _KC, 2 * B_TILE], a_dt, tag="x",
                                    name=f"x_{g}")
                nc.sync.dma_start(
                    xts[g][:, :, :n], xt_v[:, :, t * B_TILE : t * B_TILE + n]
                )

            def mm1(t):
                # per-tile L1: x slices come from the pair-granular x tile
                g, c = t // 2, t % 2
                ps1s[t] = ps1p.tile([_P, 2, B_TILE], f32, tag="ps1",
                                    name=f"ps1_{t}")
                for hh in range(2):
                    for kc in range(_KC):
                        nc.tensor.matmul(
                            ps1s[t][:, hh, :],
                            w1_sb[:, kc, hh * _P : (hh + 1) * _P],
                            xts[g][:, kc, c * B_TILE : (c + 1) * B_TILE],
                            start=(kc == 0),
                            stop=(kc == _KC - 1),
                        )
                if c == 1 or t == N_TILES - 1:
                    del xts[g]

            def silu1(t):
                h1s[t] = h1pool.tile([_P, 2, B_TILE], a_dt, tag="h1",
                                     name=f"h1_{t}")
                if zero_bias:
                    nc.scalar.activation(h1s[t][:], ps1s[t][:], SILU)
                else:
                    for hh in range(2):
                        nc.scalar.activation(
                            h1s[t][:, hh, :], ps1s[t][:, hh, :], SILU,
                            bias=b1_sb[:, hh : hh + 1],
                        )
                del ps1s[t]

            def mm2(t):
                ps2s[t] = ps2p.tile([_P, 2, B_TILE], f32, tag="ps2",
                                    name=f"ps2_{t}")
                for hh in range(2):
                    for kc in range(_KC):
                        nc.tensor.matmul(
                            ps2s[t][:, hh, :],
                            w2_sb[:, kc, hh * _P : (hh + 1) * _P],
                            h1s[t][:, kc, :],
                            start=(kc == 0),
                            stop=(kc == _KC - 1),
                        )
                del h1s[t]

            def silu2(t):
                h2s[t] = h2pool.tile([_P, 2, B_TILE], a_dt, tag="h2",
                                     name=f"h2_{t}")
                if zero_bias:
                    nc.scalar.activation(h2s[t][:], ps2s[t][:], SILU)
                else:
                    for hh in range(2):
                        nc.scalar.activation(
                            h2s[t][:, hh, :], ps2s[t][:, hh, :], SILU,
                            bias=b2_sb[:, hh : hh + 1],
                        )

            def mm3(t):
                # reuse the drained first bank of ps2s[t] as L3 psum
                for kc in range(_KC):
                    nc.tensor.matmul(
                        ps2s[t][:, 0, :],
                        w3_sb[:, kc, :],
                        h2s[t][:, kc, :],
                        start=(kc == 0),
                        stop=(kc == _KC - 1),
                    )
                del h2s[t]

            def tail(t):
                out_sb = opool.tile([_P, B_TILE], f32, tag="o", name=f"o_{t}")
                if zero_bias:
                    nc.vector.tensor_copy(out_sb[:], ps2s[t][:, 0, :])
                else:
                    nc.vector.tensor_scalar_add(
                        out_sb[:], ps2s[t][:, 0, :], b3_sb[:, 0:1]
                    )
                del ps2s[t]
                nc.sync.dma_start(
                    yt_d[:, t * B_TILE : (t + 1) * B_TILE], out_sb[:]
                )

            dma_x(0)
            load_weights()
            for t in range(2, 4):
                dma_x(t)
            mm1(0)
            silu1(0)
            for t in range(N_TILES):
                if t + 4 < N_TILES:
                    dma_x(t + 4)
                if t + 1 < N_TILES:
                    mm1(t + 1)
                if t >= 1:
                    silu2(t - 1)
                if t + 1 < N_TILES:
                    silu1(t + 1)
                mm2(t)
                if t >= 1:
                    mm3(t - 1)
                    tail(t - 1)
            silu2(N_TILES - 1)
            mm3(N_TILES - 1)
            tail(N_TILES - 1)

    nc.compile()
    return nc


def _get_program(zero_bias: bool):
    key = ("prog", zero_bias)
    if key not in _program_cache:
        _program_cache[key] = _build_program(zero_bias)
    return _program_cache[key]


def run(embedding, W1, b1, W2, b2, W3, b3, species, block_index, trace=False,
        trace_cores=None):
    """Core implementation; returns (full_output, BassKernelResults)."""
    from concourse.bass_utils import run_bass_kernel_spmd

    embedding = np.ascontiguousarray(np.asarray(embedding, dtype=np.float32))
    W1 = np.asarray(W1, dtype=np.float32)
    b1 = np.asarray(b1, dtype=np.float32)
    W2 = np.asarray(W2, dtype=np.float32)
    b2 = np.asarray(b2, dtype=np.float32)
    W3 = np.asarray(W3, dtype=np.float32)
    b3 = np.asarray(b3, dtype=np.float32)
    block_index = np.asarray(block_index)

    zero_bias = not (b1.any() or b2.any() or b3.any())
    nc = _get_program(zero_bias)
    wdt = _weight_np_dtype()
    adt = _np_dtype(ACT_DT)

    # Host-side shard: gather each species' atoms, transpose to [D_IN, BLOCK],
    # zero-pad atoms to B_PAD.
    gathered = embedding[block_index.reshape(-1)].reshape(N_SPECIES, BLOCK, D_IN)
    in_maps = []
    for s in range(N_SPECIES):
        xt = np.zeros((D_IN, B_PAD), dtype=adt)
        xt[:, :BLOCK] = gathered[s].T.astype(adt)
        m = {"xt": xt, "w1": W1[s].astype(wdt), "w2": W2[s].astype(wdt),
             "w3": W3[s].astype(wdt)}
        if not zero_bias:
            m["b1"] = b1[s]
            m["b2"] = b2[s]
            m["b3"] = b3[s]
        in_maps.append(m)

    res = run_bass_kernel_spmd(
        nc, in_maps, core_ids=list(range(N_SPECIES)), trace=trace,
        trace_cores=trace_cores,
    )

    # Unshard: un-transpose, drop padding, scatter back by block_index.
    n_out = np.asarray(species).shape[0]
    out = np.zeros((n_out, D_OUT), dtype=np.float32)
    for s in range(N_SPECIES):
        out[block_index[s]] = res.results[s]["yt"][:, :BLOCK].T
    return out, res


def kernel(**inputs) -> np.ndarray:
    out, _ = run(**inputs)
    return out
